# revision 1
# baseline (speedup 1.0000x reference)
"""Trainium2 Bass kernel for nn_AttnModule_27152783245631.

Shards batch (16) over 8 NeuronCores (2 per core). Per batch:
  pq/pk = 1x1 convs of x/y (Wq/Wk host-padded 64->128 out-channels, zeros)
  E[n,m] = exp((pk^T pq)/t)   [n on partitions, m free]
  denom[m] = sum_n E[n,m]     (all-ones lhsT matmul -> broadcast to all partitions)
  pv_T[n,c] = (Wv x)^T        (computed directly: lhsT=x, rhs=Wv^T)
  x_update[c,m] = (sum_n pv_T[n,c] E[n,m]) * (1/denom[m])
  adained = (inp - mean_i)/std_i * std_x + mean_x  (bn_stats, unbiased var + eps)
  out = LeakyReLU_0.2(conv3x3([x_update; adained], Wc) + bc)
       computed as 9 shifted matmuls over a zero-padded 34x34 SBUF image;
       LeakyReLU(t) = 0.6*t + 0.4*|t|.
Fast path assumes alpha==1, beta==1 (the provided fills); otherwise a
numpy fallback computes the exact general result on host.
"""

import numpy as np
import ml_dtypes

import concourse.bass as bass
import concourse.tile as tile
from concourse import bacc, mybir
from concourse.bass_utils import run_bass_kernel_spmd

B, C, H, W = 16, 512, 32, 32
N = H * W                  # 1024
CQ = 64
NCORES = 8
BPC = B // NCORES          # batches per core
T_INV = 1.0 / float(CQ) ** 0.5
EPS = 1e-5
NEG = 0.2
PADW = 34                  # padded spatial width
PADN = PADW * PADW         # 1156
BF16 = mybir.dt.bfloat16
F32 = mybir.dt.float32
NPBF16 = ml_dtypes.bfloat16

_prog_cache = {}


def _build(reps=1):
    nc = bacc.Bacc("TRN2", target_bir_lowering=False, debug=False)

    x16 = nc.dram_tensor("x16", [BPC, 128, 4, N], BF16, kind="ExternalInput")
    y16 = nc.dram_tensor("y16", [BPC, 128, 4, N], BF16, kind="ExternalInput")
    i16 = nc.dram_tensor("i16", [BPC, 128, 4, N], BF16, kind="ExternalInput")
    wq = nc.dram_tensor("wq", [128, 4, 128], BF16, kind="ExternalInput")
    wk = nc.dram_tensor("wk", [128, 4, 128], BF16, kind="ExternalInput")
    wv = nc.dram_tensor("wv", [128, 4, 512], BF16, kind="ExternalInput")
    wc = nc.dram_tensor("wc", [128, 72, 512], BF16, kind="ExternalInput")
    bqk = nc.dram_tensor("bqk", [128, 2], F32, kind="ExternalInput")
    bvv = nc.dram_tensor("bvv", [512], F32, kind="ExternalInput")
    bcc = nc.dram_tensor("bcc", [128, 4], F32, kind="ExternalInput")
    o32 = nc.dram_tensor("o32", [BPC, 4, 128, N], F32, kind="ExternalOutput")

    with tile.TileContext(nc) as tc:
        with tc.tile_pool(name="consts", bufs=1) as consts, \
             tc.tile_pool(name="io", bufs=2) as io, \
             tc.tile_pool(name="work", bufs=1) as work, \
             tc.tile_pool(name="small", bufs=2) as small, \
             tc.tile_pool(name="ostage", bufs=3) as ostage, \
             tc.tile_pool(name="ps", bufs=3, space="PSUM") as ps, \
             tc.tile_pool(name="psd", bufs=2, space="PSUM") as psd, \
             tc.tile_pool(name="psc", bufs=3, space="PSUM") as psc:

            # ---- constants (small ones first; big weights on the gpsimd
            #      queue so they don't stall the activation loads) ----
            wq_sb = consts.tile([128, 4, 128], BF16)
            nc.sync.dma_start(wq_sb[:], wq[:])
            wk_sb = consts.tile([128, 4, 128], BF16)
            nc.sync.dma_start(wk_sb[:], wk[:])
            bqk_sb = consts.tile([128, 2], F32)
            nc.sync.dma_start(bqk_sb[:], bqk[:])
            bcc_sb = consts.tile([128, 4], F32)
            nc.sync.dma_start(bcc_sb[:], bcc[:])
            bcc04_sb = consts.tile([128, 4], F32)
            nc.vector.tensor_scalar_mul(bcc04_sb[:], bcc_sb[:], 0.4)
            bv_sb = consts.tile([128, 512], F32)
            nc.sync.dma_start(bv_sb[:], bvv[None, :].to_broadcast((128, 512)))
            ones_sb = consts.tile([128, 128], BF16)
            nc.vector.memset(ones_sb[:], 1.0)
            eps_sb = consts.tile([128, 1], F32)
            nc.vector.memset(eps_sb[:], EPS)
            pad_sb = consts.tile([128, 8, PADN], BF16)
            nc.gpsimd.memset(pad_sb[:], 0.0)
            pad_v = pad_sb.rearrange("p k (a b) -> p k a b", a=PADW)
            wv_sb = consts.tile([128, 4, 512], BF16)
            nc.gpsimd.dma_start(wv_sb[:], wv[:])
            wc_sb = consts.tile([128, 72, 512], BF16)
            for wkt in range(8):
                nc.gpsimd.dma_start(wc_sb[:, bass.ts(wkt, 9), :],
                                    wc[:, bass.ts(wkt, 9), :])

            for _ in range(reps):
                for b in range(BPC):
                    # ---- load activations (split per ktile so consumers
                    #      can start on the first chunk) ----
                    x_sb = io.tile([128, 4, N], BF16, tag="x_sb")
                    y_sb = io.tile([128, 4, N], BF16, tag="y_sb")
                    i_sb = io.tile([128, 4, N], BF16, tag="i_sb")
                    for kt in range(4):
                        nc.sync.dma_start(x_sb[:, kt, :], x16[b, :, kt, :])
                    for kt in range(4):
                        nc.sync.dma_start(y_sb[:, kt, :], y16[b, :, kt, :])
                    for kt in range(4):
                        nc.sync.dma_start(i_sb[:, kt, :], i16[b, :, kt, :])

                    # ---- pq = Wq x + bq ; pk = Wk y + bk  (rows 64..127 zero) ----
                    pq_sb = work.tile([128, N], BF16, tag="pq")
                    pk_sb = work.tile([128, N], BF16, tag="pk")
                    for (dst, wsb, src, bcol) in ((pq_sb, wq_sb, x_sb, 0),
                                                  (pk_sb, wk_sb, y_sb, 1)):
                        for mc in range(4):
                            pt = ps.tile([128, 256], F32, tag="acc")
                            for kt in range(4):
                                nc.tensor.matmul(
                                    pt[:], wsb[:, kt, :],
                                    src[:, kt, bass.ts(mc, 256)],
                                    start=(kt == 0), stop=(kt == 3))
                            nc.vector.tensor_scalar(
                                dst[:, bass.ts(mc, 256)], pt[:],
                                bqk_sb[:, bcol:bcol + 1], None,
                                op0=mybir.AluOpType.add)

                    # ---- pv_T[n, c] = (Wv x + bv)^T ----
                    pvT_sb = work.tile([128, 8, 512], BF16, tag="pvT")
                    for nt in range(8):
                        for ch in range(2):
                            pt = ps.tile([128, 256], F32, tag="acc")
                            for kt in range(4):
                                nc.tensor.matmul(
                                    pt[:], x_sb[:, kt, bass.ts(nt, 128)],
                                    wv_sb[:, kt, bass.ts(ch, 256)],
                                    start=(kt == 0), stop=(kt == 3))
                            nc.vector.tensor_tensor(
                                pvT_sb[:, nt, bass.ts(ch, 256)], pt[:],
                                bv_sb[:, bass.ts(ch, 256)],
                                mybir.AluOpType.add)

                    # ---- E = exp(S^T / t), denom broadcast ----
                    e_sb = work.tile([128, 8, N], BF16, tag="e_sb")
                    den_ps = [psd.tile([128, 512], F32, tag="den_ps",
                                       name=f"den_ps_{mc}")
                              for mc in range(2)]
                    for nt in range(8):
                        for mh in range(2):
                            et = ps.tile([128, 512], F32, tag="acc")
                            for q in range(2):
                                nc.tensor.matmul(
                                    et[:, bass.ts(q, 256)],
                                    pk_sb[:, bass.ts(nt, 128)],
                                    pq_sb[:, bass.ts(mh * 2 + q, 256)],
                                    start=True, stop=True)
                            nc.scalar.activation(
                                e_sb[:, nt, bass.ts(mh, 512)], et[:],
                                mybir.ActivationFunctionType.Exp,
                                bias=0.0, scale=T_INV)
                        if nt >= 1:
                            for mh in range(2):
                                nc.tensor.matmul(
                                    den_ps[mh][:], ones_sb[:],
                                    e_sb[:, nt - 1, bass.ts(mh, 512)],
                                    start=(nt == 1), stop=False)
                    for mh in range(2):
                        nc.tensor.matmul(
                            den_ps[mh][:], ones_sb[:],
                            e_sb[:, 7, bass.ts(mh, 512)],
                            start=False, stop=True)
                    rec_sb = work.tile([128, N], F32, tag="rec")
                    for mc in range(2):
                        nc.vector.reciprocal(rec_sb[:, bass.ts(mc, 512)],
                                             den_ps[mc][:])
                    rec_v = rec_sb.rearrange("p (a b) -> p a b", a=32)

                    # ---- x_update -> pad rows 0..3 ----
                    for ct in range(4):
                        for mc in range(4):
                            xt = ps.tile([128, 256], F32, tag="acc")
                            for nt in range(8):
                                nc.tensor.matmul(
                                    xt[:], pvT_sb[:, nt, bass.ts(ct, 128)],
                                    e_sb[:, nt, bass.ts(mc, 256)],
                                    start=(nt == 0), stop=(nt == 7))
                            nc.vector.tensor_tensor(
                                pad_v[:, ct, 1 + mc * 8:1 + mc * 8 + 8, 1:33],
                                xt[:].rearrange("p (a b) -> p a b", a=8),
                                rec_v[:, mc * 8:mc * 8 + 8, :],
                                mybir.AluOpType.mult)

                    # ---- AdaIN -> pad rows 4..7 ----
                    for ct in range(4):
                        stx = small.tile([128, 2, 6], F32, tag="stx")
                        nc.vector.bn_stats(out=stx[:, 0, :], in_=x_sb[:, ct, 0:512])
                        nc.vector.bn_stats(out=stx[:, 1, :], in_=x_sb[:, ct, 512:N])
                        mvx = small.tile([128, 2], F32, tag="mvx")
                        nc.vector.bn_aggr(out=mvx[:], in_=stx[:])
                        sti = small.tile([128, 2, 6], F32, tag="sti")
                        nc.vector.bn_stats(out=sti[:, 0, :], in_=i_sb[:, ct, 0:512])
                        nc.vector.bn_stats(out=sti[:, 1, :], in_=i_sb[:, ct, 512:N])
                        mvi = small.tile([128, 2], F32, tag="mvi")
                        nc.vector.bn_aggr(out=mvi[:], in_=sti[:])
                        # std = sqrt(var_pop * n/(n-1) + eps)
                        tstd = small.tile([128, 1], F32, tag="tstd")
                        nc.scalar.activation(
                            tstd[:], mvx[:, 1:2],
                            mybir.ActivationFunctionType.Sqrt,
                            bias=eps_sb[:], scale=float(N) / (N - 1))
                        istd = small.tile([128, 1], F32, tag="istd")
                        nc.scalar.activation(
                            istd[:], mvi[:, 1:2],
                            mybir.ActivationFunctionType.Sqrt,
                            bias=eps_sb[:], scale=float(N) / (N - 1))
                        irstd = small.tile([128, 1], F32, tag="irstd")
                        nc.vector.reciprocal(irstd[:], istd[:])
                        scale = small.tile([128, 1], F32, tag="scale")
                        nc.vector.tensor_tensor(scale[:], tstd[:], irstd[:],
                                                mybir.AluOpType.mult)
                        shift = small.tile([128, 1], F32, tag="shift")
                        nc.vector.tensor_tensor(shift[:], mvi[:, 0:1], scale[:],
                                                mybir.AluOpType.mult)
                        nc.vector.tensor_tensor(shift[:], mvx[:, 0:1], shift[:],
                                                mybir.AluOpType.subtract)
                        nc.vector.tensor_scalar(
                            pad_v[:, 4 + ct, 1:33, 1:33],
                            i_sb[:, ct].rearrange("p (a b) -> p a b", a=32),
                            scale[:], shift[:],
                            op0=mybir.AluOpType.mult, op1=mybir.AluOpType.add)

                    # ---- conv 3x3 + LeakyReLU (N=256 chunks of 8 rows) ----
                    ntaps = 9
                    for mt in range(4):
                        for hq in range(4):
                            pc = psc.tile([128, 256], F32, tag="conv_ps")
                            for tap in range(ntaps):
                                ky, kx = tap // 3, tap % 3
                                for kt in range(8):
                                    nc.tensor.matmul(
                                        pc[:],
                                        wc_sb[:, tap * 8 + kt, bass.ts(mt, 128)],
                                        pad_v[:, kt,
                                              hq * 8 + ky:hq * 8 + ky + 8,
                                              kx:kx + 32],
                                        start=(tap == 0 and kt == 0),
                                        stop=(tap == ntaps - 1 and kt == 7))
                            ab = ostage.tile([128, 256], F32, tag="ab")
                            nc.scalar.activation(
                                ab[:], pc[:],
                                mybir.ActivationFunctionType.Abs,
                                bias=bcc04_sb[:, mt:mt + 1], scale=0.4)
                            ot = ostage.tile([128, 256], F32, tag="ot")
                            nc.vector.tensor_scalar(
                                ot[:], pc[:],
                                bcc_sb[:, mt:mt + 1], 0.6,
                                op0=mybir.AluOpType.add,
                                op1=mybir.AluOpType.mult)
                            nc.vector.tensor_tensor(ot[:], ot[:], ab[:],
                                                    mybir.AluOpType.add)
                            nc.sync.dma_start(
                                o32[b, mt, :, bass.ts(hq, 256)], ot[:])

    nc.finalize()
    return nc


def _get_prog(reps=1):
    if reps not in _prog_cache:
        _prog_cache[reps] = _build(reps)
    return _prog_cache[reps]


def _prep_in_maps(inp, x, y, Wq, bq, Wk, bk, Wv, bv, Wc, bc):
    def act_prep(a):
        # [B,C,H,W] f32 -> [B,128,4,N] bf16 (c = ct*128 + p)
        return np.ascontiguousarray(
            a.reshape(B, 4, 128, N).transpose(0, 2, 1, 3)).astype(NPBF16)

    x16 = act_prep(x)
    y16 = act_prep(y)
    i16 = act_prep(inp)

    def wqk_prep(w):
        wp = np.zeros((128, C), np.float32)
        wp[:CQ] = w
        return np.ascontiguousarray(
            wp.T.reshape(4, 128, 128).transpose(1, 0, 2)).astype(NPBF16)

    wq_h = wqk_prep(Wq)
    wk_h = wqk_prep(Wk)
    wv_h = np.ascontiguousarray(
        Wv.T.reshape(4, 128, 512).transpose(1, 0, 2)).astype(NPBF16)
    wc_h = np.ascontiguousarray(
        Wc.transpose(2, 3, 1, 0).reshape(9, 8, 128, 512)
        .transpose(2, 0, 1, 3).reshape(128, 72, 512)).astype(NPBF16)
    bqk_h = np.zeros((128, 2), np.float32)
    bqk_h[:CQ, 0] = bq
    bqk_h[:CQ, 1] = bk
    bvv_h = bv.astype(np.float32)
    bcc_h = np.ascontiguousarray(bc.reshape(4, 128).T).astype(np.float32)

    in_maps = []
    for c in range(NCORES):
        s = slice(c * BPC, (c + 1) * BPC)
        in_maps.append({
            "x16": x16[s], "y16": y16[s], "i16": i16[s],
            "wq": wq_h, "wk": wk_h, "wv": wv_h, "wc": wc_h,
            "bqk": bqk_h, "bvv": bvv_h, "bcc": bcc_h,
        })
    return in_maps


def _assemble(results):
    out = np.empty((B, C, H, W), np.float32)
    for c in range(NCORES):
        o = results[c]["o32"]  # [BPC, 4, 128, N]
        out[c * BPC:(c + 1) * BPC] = o.reshape(BPC, C, H, W)
    return out


def _np_reference(inp, x, y, Wq, bq, Wk, bk, Wv, bv, Wc, bc, alpha, beta):
    # Exact general-path fallback on host (numpy, fp32).
    b, c, h, w = x.shape
    n = h * w
    t = float(CQ) ** 0.5

    def conv1x1(a, Wm, bb):
        return (np.einsum("oc,bcn->bon", Wm, a.reshape(b, c, n))
                + bb[None, :, None])

    def softmax(s):
        s = s - s.max(axis=-1, keepdims=True)
        e = np.exp(s)
        return e / e.sum(axis=-1, keepdims=True)

    pq = conv1x1(x, Wq, bq)
    pk = conv1x1(y, Wk, bk)
    pv = conv1x1(x, Wv, bv)
    attn_iden = softmax(np.einsum("bcn,bcm->bnm", pq, pk) / t)
    pq_p = conv1x1(y, Wq, bq)
    pk_p = conv1x1(x, Wk, bk)
    attn_pose = softmax(np.einsum("bcn,bcm->bnm", pq_p, pk_p) / t)
    xu = np.einsum("bcn,bmn->bcm", pv, attn_iden).reshape(b, c, h, w)
    xu = (1.0 - beta) * x + beta * xu

    def mean_std(f):
        v = f.reshape(b, c, n)
        m = v.mean(axis=2)
        s = np.sqrt(v.var(axis=2, ddof=1) + EPS)
        return m[:, :, None], s[:, :, None]

    tm, ts_ = mean_std(x)
    im, is_ = mean_std(inp)
    ad = ((inp.reshape(b, c, n) - im) / is_ * ts_ + tm)
    rev = np.einsum("bcn,bmn->bcm", ad, 1.0 - attn_pose)
    rev = (1.0 - alpha) * rev + alpha * ad
    cat = np.concatenate([xu.reshape(b, c, n), rev], axis=1).reshape(
        b, 2 * c, h, w)
    catp = np.pad(cat, ((0, 0), (0, 0), (1, 1), (1, 1)))
    out = np.zeros((b, c, h, w), np.float32)
    for ky in range(3):
        for kx in range(3):
            out += np.einsum("oi,bihw->bohw", Wc[:, :, ky, kx],
                             catp[:, :, ky:ky + h, kx:kx + w])
    out += bc[None, :, None, None]
    return np.where(out >= 0, out, NEG * out).astype(np.float32)


def _run(in_maps, reps=1):
    nc = _get_prog(reps)
    return run_bass_kernel_spmd(nc, in_maps, list(range(NCORES)))


def kernel(inp, x, y, Wq, bq, Wk, bk, Wv, bv, Wc, bc, alpha, beta):
    args = [np.asarray(a, np.float32) for a in
            (inp, x, y, Wq, bq, Wk, bk, Wv, bv, Wc, bc)]
    alpha = np.asarray(alpha, np.float32)
    beta = np.asarray(beta, np.float32)
    if float(alpha.reshape(-1)[0]) != 1.0 or float(beta.reshape(-1)[0]) != 1.0:
        return _np_reference(*args, alpha.reshape(-1)[0], beta.reshape(-1)[0])
    in_maps = _prep_in_maps(*args)
    res = _run(in_maps)
    return _assemble(res.results)



# revision 2
# speedup vs baseline: 1.8483x; 1.8483x over previous
"""Trainium2 Bass kernel for nn_AttnModule_27152783245631.

Shards batch (16) over 8 NeuronCores (2 per core). fp8e4 (DoubleRow,
K=256 per matmul) is used everywhere precision allows:
  pq = (32Wq)8 x8 / 32 + bq        pk = (32Wk)8 y8 / 32 + bk     (bf16 out)
  pvT8 = fp8(32 (Wv x + bv))       (DoubleRow fp8 matmul, K-pairs)
  E8[n,m] = fp8(exp(pk^T pq / t))  (S matmul in bf16)
  den[m] = sum_n E8  (all-ones fp8 DoubleRow matmul); rec = 1/den
  pad_xu8 = fp8((pvT8 E8) * rec) = fp8(32 xu)  (DoubleRow matmul)
  adained (AdaIN via bn_stats, bf16) -> pad_ad
  conv3x3: adained half in bf16 (9x4 matmuls/chunk, 32-wide rows);
           xu half in fp8 DoubleRow over full 34-wide padded rows
           (contiguous F=272, junk columns discarded); scales: weights
           x8, activations x32 -> psum/256.
  out = LeakyReLU_0.2(pcA + pcX/256 + bc) via 0.6t + 0.4|t|.
Fast path assumes alpha==1, beta==1 (the provided fills); otherwise a
numpy fallback computes the exact general result on host.
"""

import numpy as np
import ml_dtypes

import concourse.bass as bass
import concourse.tile as tile
from concourse import bacc, mybir
from concourse.bass_utils import run_bass_kernel_spmd

B, C, H, W = 16, 512, 32, 32
N = H * W                  # 1024
CQ = 64
NCORES = 8
BPC = B // NCORES          # batches per core
T_INV = 1.0 / float(CQ) ** 0.5
EPS = 1e-5
NEG = 0.2
PADW = 34                  # padded spatial width
PADN = PADW * PADW         # 1156
BF16 = mybir.dt.bfloat16
FP8 = mybir.dt.float8e4
F32 = mybir.dt.float32
NPBF16 = ml_dtypes.bfloat16
NPFP8 = ml_dtypes.float8_e4m3
DR = mybir.MatmulPerfMode.DoubleRow

_prog_cache = {}


def _build(reps=1):
    nc = bacc.Bacc("TRN2", target_bir_lowering=False, debug=False)

    x16 = nc.dram_tensor("x16", [BPC, 128, 4, N], BF16, kind="ExternalInput")
    i16 = nc.dram_tensor("i16", [BPC, 128, 4, N], BF16, kind="ExternalInput")
    x8 = nc.dram_tensor("x8", [BPC, 128, 4, N], FP8, kind="ExternalInput")
    y8 = nc.dram_tensor("y8", [BPC, 128, 4, N], FP8, kind="ExternalInput")
    wq = nc.dram_tensor("wq", [128, 4, 128], FP8, kind="ExternalInput")
    wk = nc.dram_tensor("wk", [128, 4, 128], FP8, kind="ExternalInput")
    wv = nc.dram_tensor("wv", [128, 4, 512], FP8, kind="ExternalInput")
    wcad = nc.dram_tensor("wcad", [128, 36, 512], BF16, kind="ExternalInput")
    wcxu = nc.dram_tensor("wcxu", [128, 36, 512], FP8, kind="ExternalInput")
    bqk = nc.dram_tensor("bqk", [128, 2], F32, kind="ExternalInput")
    bvv = nc.dram_tensor("bvv", [512], F32, kind="ExternalInput")
    bcc = nc.dram_tensor("bcc", [128, 4], F32, kind="ExternalInput")
    o32 = nc.dram_tensor("o32", [BPC, 4, 128, N], F32, kind="ExternalOutput")

    with tile.TileContext(nc) as tc:
        with tc.tile_pool(name="consts", bufs=1) as consts, \
             tc.tile_pool(name="io", bufs=2) as io, \
             tc.tile_pool(name="work", bufs=1) as work, \
             tc.tile_pool(name="small", bufs=2) as small, \
             tc.tile_pool(name="ostage", bufs=3) as ostage, \
             tc.tile_pool(name="ps", bufs=3, space="PSUM") as ps, \
             tc.tile_pool(name="psc", bufs=2, space="PSUM") as psc, \
             tc.tile_pool(name="psx", bufs=2, space="PSUM") as psx:

            # ---- constants (small ones first; big weights on the gpsimd
            #      queue so they don't stall the activation loads) ----
            wq_sb = consts.tile([128, 4, 128], FP8)
            nc.sync.dma_start(wq_sb[:], wq[:])
            wk_sb = consts.tile([128, 4, 128], FP8)
            nc.sync.dma_start(wk_sb[:], wk[:])
            bqk_sb = consts.tile([128, 2], F32)
            nc.sync.dma_start(bqk_sb[:], bqk[:])
            bcc_sb = consts.tile([128, 4], F32)
            nc.sync.dma_start(bcc_sb[:], bcc[:])
            bv32_sb = consts.tile([128, 512], F32)
            nc.sync.dma_start(bv32_sb[:], bvv[None, :].to_broadcast((128, 512)))
            ones_sb = consts.tile([128, 2, 128], FP8)
            nc.vector.memset(ones_sb[:], 1.0)
            eps_sb = consts.tile([128, 1], F32)
            nc.vector.memset(eps_sb[:], EPS)
            pad_ad = consts.tile([128, 4, PADN], BF16)
            nc.gpsimd.memset(pad_ad[:], 0.0)
            pad_ad_v = pad_ad.rearrange("p k (a b) -> p k a b", a=PADW)
            pad_xu = consts.tile([128, 4, PADN + 2], FP8)
            nc.gpsimd.memset(pad_xu[:], 0.0)
            pad_xu_v = pad_xu[:, :, :PADN].rearrange("p k (a b) -> p k a b",
                                                     a=PADW)
            wv_sb = consts.tile([128, 4, 512], FP8)
            nc.gpsimd.dma_start(wv_sb[:], wv[:])
            wcad_sb = consts.tile([128, 36, 512], BF16)
            for wkt in range(4):
                nc.gpsimd.dma_start(wcad_sb[:, bass.ts(wkt, 9), :],
                                    wcad[:, bass.ts(wkt, 9), :])
            wcxu_sb = consts.tile([128, 36, 512], FP8)
            for wkt in range(4):
                nc.gpsimd.dma_start(wcxu_sb[:, bass.ts(wkt, 9), :],
                                    wcxu[:, bass.ts(wkt, 9), :])

            for _ in range(reps):
                for b in range(BPC):
                    # ---- load activations ----
                    x_sb = io.tile([128, 4, N], BF16, tag="x_sb")
                    i_sb = io.tile([128, 4, N], BF16, tag="i_sb")
                    x8_sb = io.tile([128, 4, N], FP8, tag="x8_sb")
                    y8_sb = io.tile([128, 4, N], FP8, tag="y8_sb")
                    for kt in range(4):
                        nc.sync.dma_start(x8_sb[:, kt, :], x8[b, :, kt, :])
                    for kt in range(4):
                        nc.sync.dma_start(y8_sb[:, kt, :], y8[b, :, kt, :])
                    for kt in range(4):
                        nc.sync.dma_start(x_sb[:, kt, :], x16[b, :, kt, :])
                    for kt in range(4):
                        nc.sync.dma_start(i_sb[:, kt, :], i16[b, :, kt, :])

                    # ---- pq = Wq x + bq ; pk = Wk y + bk (rows 64.. zero) ----
                    pq_sb = work.tile([128, N], BF16, tag="pq")
                    pk_sb = work.tile([128, N], BF16, tag="pk")
                    for (dst, wsb, src, bcol) in ((pq_sb, wq_sb, x8_sb, 0),
                                                  (pk_sb, wk_sb, y8_sb, 1)):
                        for mc in range(4):
                            pt = ps.tile([128, 256], F32, tag="acc")
                            for p in range(2):
                                nc.tensor.matmul(
                                    pt[:], wsb[:, 2 * p:2 * p + 2, :],
                                    src[:, 2 * p:2 * p + 2, bass.ts(mc, 256)],
                                    start=(p == 0), stop=(p == 1),
                                    perf_mode=DR)
                            nc.vector.tensor_scalar(
                                dst[:, bass.ts(mc, 256)], pt[:],
                                1.0 / 32.0, bqk_sb[:, bcol:bcol + 1],
                                op0=mybir.AluOpType.mult,
                                op1=mybir.AluOpType.add)

                    # ---- pvT8[n, c] = fp8(32 (Wv x + bv))^T ----
                    pvT_sb = work.tile([128, 8, 512], FP8, tag="pvT")
                    for nt in range(8):
                        for ch in range(2):
                            pt = ps.tile([128, 256], F32, tag="acc")
                            for p in range(2):
                                nc.tensor.matmul(
                                    pt[:],
                                    x8_sb[:, 2 * p:2 * p + 2, bass.ts(nt, 128)],
                                    wv_sb[:, 2 * p:2 * p + 2, bass.ts(ch, 256)],
                                    start=(p == 0), stop=(p == 1),
                                    perf_mode=DR)
                            nc.vector.tensor_tensor(
                                pvT_sb[:, nt, bass.ts(ch, 256)], pt[:],
                                bv32_sb[:, bass.ts(ch, 256)],
                                mybir.AluOpType.add)

                    # ---- E8 = fp8(exp(S / t)) ----
                    e_sb = work.tile([128, 8, N], FP8, tag="e_sb")
                    for nt in range(8):
                        for mh in range(2):
                            et = ps.tile([128, 512], F32, tag="acc")
                            for q in range(2):
                                nc.tensor.matmul(
                                    et[:, bass.ts(q, 256)],
                                    pk_sb[:, bass.ts(nt, 128)],
                                    pq_sb[:, bass.ts(mh * 2 + q, 256)],
                                    start=True, stop=True)
                            nc.scalar.activation(
                                e_sb[:, nt, bass.ts(mh, 512)], et[:],
                                mybir.ActivationFunctionType.Exp,
                                bias=0.0, scale=T_INV)

                    # ---- den = sum_n E8 (broadcast), rec = 1/den ----
                    rec_sb = work.tile([128, N], F32, tag="rec")
                    for mc in range(4):
                        dt_ps = ps.tile([128, 256], F32, tag="acc")
                        for p in range(4):
                            nc.tensor.matmul(
                                dt_ps[:], ones_sb[:],
                                e_sb[:, 2 * p:2 * p + 2, bass.ts(mc, 256)],
                                start=(p == 0), stop=(p == 3),
                                perf_mode=DR)
                        nc.vector.reciprocal(rec_sb[:, bass.ts(mc, 256)],
                                             dt_ps[:])
                    rec_v = rec_sb.rearrange("p (a b) -> p a b", a=32)

                    # ---- x_update*32 -> fp8 pad rows ----
                    for ct in range(4):
                        for mc in range(4):
                            xt = ps.tile([128, 256], F32, tag="acc")
                            for p in range(4):
                                nc.tensor.matmul(
                                    xt[:],
                                    pvT_sb[:, 2 * p:2 * p + 2, bass.ts(ct, 128)],
                                    e_sb[:, 2 * p:2 * p + 2, bass.ts(mc, 256)],
                                    start=(p == 0), stop=(p == 3),
                                    perf_mode=DR)
                            nc.vector.tensor_tensor(
                                pad_xu_v[:, ct, 1 + mc * 8:1 + mc * 8 + 8, 1:33],
                                xt[:].rearrange("p (a b) -> p a b", a=8),
                                rec_v[:, mc * 8:mc * 8 + 8, :],
                                mybir.AluOpType.mult)

                    # ---- AdaIN -> bf16 pad rows ----
                    for ct in range(4):
                        stx = small.tile([128, 2, 6], F32, tag="stx")
                        nc.vector.bn_stats(out=stx[:, 0, :], in_=x_sb[:, ct, 0:512])
                        nc.vector.bn_stats(out=stx[:, 1, :], in_=x_sb[:, ct, 512:N])
                        mvx = small.tile([128, 2], F32, tag="mvx")
                        nc.vector.bn_aggr(out=mvx[:], in_=stx[:])
                        sti = small.tile([128, 2, 6], F32, tag="sti")
                        nc.vector.bn_stats(out=sti[:, 0, :], in_=i_sb[:, ct, 0:512])
                        nc.vector.bn_stats(out=sti[:, 1, :], in_=i_sb[:, ct, 512:N])
                        mvi = small.tile([128, 2], F32, tag="mvi")
                        nc.vector.bn_aggr(out=mvi[:], in_=sti[:])
                        # std = sqrt(var_pop * n/(n-1) + eps)
                        tstd = small.tile([128, 1], F32, tag="tstd")
                        nc.scalar.activation(
                            tstd[:], mvx[:, 1:2],
                            mybir.ActivationFunctionType.Sqrt,
                            bias=eps_sb[:], scale=float(N) / (N - 1))
                        istd = small.tile([128, 1], F32, tag="istd")
                        nc.scalar.activation(
                            istd[:], mvi[:, 1:2],
                            mybir.ActivationFunctionType.Sqrt,
                            bias=eps_sb[:], scale=float(N) / (N - 1))
                        irstd = small.tile([128, 1], F32, tag="irstd")
                        nc.vector.reciprocal(irstd[:], istd[:])
                        scale = small.tile([128, 1], F32, tag="scale")
                        nc.vector.tensor_tensor(scale[:], tstd[:], irstd[:],
                                                mybir.AluOpType.mult)
                        shift = small.tile([128, 1], F32, tag="shift")
                        nc.vector.tensor_tensor(shift[:], mvi[:, 0:1], scale[:],
                                                mybir.AluOpType.mult)
                        nc.vector.tensor_tensor(shift[:], mvx[:, 0:1], shift[:],
                                                mybir.AluOpType.subtract)
                        nc.vector.tensor_scalar(
                            pad_ad_v[:, ct, 1:33, 1:33],
                            i_sb[:, ct].rearrange("p (a b) -> p a b", a=32),
                            scale[:], shift[:],
                            op0=mybir.AluOpType.mult, op1=mybir.AluOpType.add)

                    # ---- conv 3x3 + LeakyReLU ----
                    for mt in range(4):
                        for hq in range(4):
                            pca = psc.tile([128, 256], F32, tag="conv_ps")
                            for tap in range(9):
                                ky, kx = tap // 3, tap % 3
                                for j in range(4):
                                    nc.tensor.matmul(
                                        pca[:],
                                        wcad_sb[:, tap * 4 + j, bass.ts(mt, 128)],
                                        pad_ad_v[:, j,
                                                 hq * 8 + ky:hq * 8 + ky + 8,
                                                 kx:kx + 32],
                                        start=(tap == 0 and j == 0),
                                        stop=(tap == 8 and j == 3))
                            pcx = psx.tile([128, 272], F32, tag="convx_ps")
                            for tap in range(9):
                                ky, kx = tap // 3, tap % 3
                                off = (hq * 8 + ky) * PADW + kx
                                for p in range(2):
                                    nc.tensor.matmul(
                                        pcx[:],
                                        wcxu_sb[:, tap * 4 + 2 * p:tap * 4 + 2 * p + 2,
                                                bass.ts(mt, 128)],
                                        pad_xu[:, 2 * p:2 * p + 2, off:off + 272],
                                        start=(tap == 0 and p == 0),
                                        stop=(tap == 8 and p == 1),
                                        perf_mode=DR)
                            u = ostage.tile([128, 256], F32, tag="u")
                            nc.vector.tensor_scalar(
                                u[:].rearrange("p (a b) -> p a b", a=8),
                                pcx[:].rearrange("p (a b) -> p a b", a=8)[:, :, 0:32],
                                1.0 / 256.0, bcc_sb[:, mt:mt + 1],
                                op0=mybir.AluOpType.mult,
                                op1=mybir.AluOpType.add)
                            tfull = ostage.tile([128, 256], F32, tag="tfull")
                            nc.vector.tensor_tensor(tfull[:], u[:], pca[:],
                                                    mybir.AluOpType.add)
                            ab = ostage.tile([128, 256], F32, tag="ab")
                            nc.scalar.activation(
                                ab[:], tfull[:],
                                mybir.ActivationFunctionType.Abs,
                                bias=0.0, scale=0.4)
                            ot = ostage.tile([128, 256], F32, tag="ot")
                            nc.scalar.activation(
                                ot[:], tfull[:],
                                mybir.ActivationFunctionType.Copy,
                                bias=0.0, scale=0.6)
                            res = ostage.tile([128, 256], F32, tag="res")
                            nc.vector.tensor_tensor(res[:], ot[:], ab[:],
                                                    mybir.AluOpType.add)
                            nc.sync.dma_start(
                                o32[b, mt, :, bass.ts(hq, 256)], res[:])

    nc.finalize()
    return nc


def _get_prog(reps=1):
    if reps not in _prog_cache:
        _prog_cache[reps] = _build(reps)
    return _prog_cache[reps]


def _prep_in_maps(inp, x, y, Wq, bq, Wk, bk, Wv, bv, Wc, bc):
    def act_prep(a, dt):
        # [B,C,H,W] f32 -> [B,128,4,N] (c = ct*128 + p)
        return np.ascontiguousarray(
            a.reshape(B, 4, 128, N).transpose(0, 2, 1, 3)).astype(dt)

    x16 = act_prep(x, NPBF16)
    i16 = act_prep(inp, NPBF16)
    x8 = act_prep(x, NPFP8)
    y8 = act_prep(y, NPFP8)

    def wqk_prep(w):
        wp = np.zeros((128, C), np.float32)
        wp[:CQ] = 32.0 * w
        return np.ascontiguousarray(
            wp.T.reshape(4, 128, 128).transpose(1, 0, 2)).astype(NPFP8)

    wq_h = wqk_prep(Wq)
    wk_h = wqk_prep(Wk)
    wv_h = np.ascontiguousarray(
        (32.0 * Wv).T.reshape(4, 128, 512).transpose(1, 0, 2)).astype(NPFP8)
    wtap = Wc.transpose(2, 3, 1, 0).reshape(9, 8, 128, 512)
    wcad_h = np.ascontiguousarray(
        wtap[:, 4:8].transpose(2, 0, 1, 3).reshape(128, 36, 512)).astype(NPBF16)
    wcxu_h = np.ascontiguousarray(
        (8.0 * wtap[:, 0:4]).transpose(2, 0, 1, 3).reshape(128, 36, 512)
    ).astype(NPFP8)
    bqk_h = np.zeros((128, 2), np.float32)
    bqk_h[:CQ, 0] = bq
    bqk_h[:CQ, 1] = bk
    bvv_h = (32.0 * bv).astype(np.float32)
    bcc_h = np.ascontiguousarray(bc.reshape(4, 128).T).astype(np.float32)

    in_maps = []
    for c in range(NCORES):
        s = slice(c * BPC, (c + 1) * BPC)
        in_maps.append({
            "x16": x16[s], "i16": i16[s], "x8": x8[s], "y8": y8[s],
            "wq": wq_h, "wk": wk_h, "wv": wv_h,
            "wcad": wcad_h, "wcxu": wcxu_h,
            "bqk": bqk_h, "bvv": bvv_h, "bcc": bcc_h,
        })
    return in_maps


def _assemble(results):
    out = np.empty((B, C, H, W), np.float32)
    for c in range(NCORES):
        o = results[c]["o32"]  # [BPC, 4, 128, N]
        out[c * BPC:(c + 1) * BPC] = o.reshape(BPC, C, H, W)
    return out


def _np_reference(inp, x, y, Wq, bq, Wk, bk, Wv, bv, Wc, bc, alpha, beta):
    # Exact general-path fallback on host (numpy, fp32).
    b, c, h, w = x.shape
    n = h * w
    t = float(CQ) ** 0.5

    def conv1x1(a, Wm, bb):
        return (np.einsum("oc,bcn->bon", Wm, a.reshape(b, c, n))
                + bb[None, :, None])

    def softmax(s):
        s = s - s.max(axis=-1, keepdims=True)
        e = np.exp(s)
        return e / e.sum(axis=-1, keepdims=True)

    pq = conv1x1(x, Wq, bq)
    pk = conv1x1(y, Wk, bk)
    pv = conv1x1(x, Wv, bv)
    attn_iden = softmax(np.einsum("bcn,bcm->bnm", pq, pk) / t)
    pq_p = conv1x1(y, Wq, bq)
    pk_p = conv1x1(x, Wk, bk)
    attn_pose = softmax(np.einsum("bcn,bcm->bnm", pq_p, pk_p) / t)
    xu = np.einsum("bcn,bmn->bcm", pv, attn_iden).reshape(b, c, h, w)
    xu = (1.0 - beta) * x + beta * xu

    def mean_std(f):
        v = f.reshape(b, c, n)
        m = v.mean(axis=2)
        s = np.sqrt(v.var(axis=2, ddof=1) + EPS)
        return m[:, :, None], s[:, :, None]

    tm, ts_ = mean_std(x)
    im, is_ = mean_std(inp)
    ad = ((inp.reshape(b, c, n) - im) / is_ * ts_ + tm)
    rev = np.einsum("bcn,bmn->bcm", ad, 1.0 - attn_pose)
    rev = (1.0 - alpha) * rev + alpha * ad
    cat = np.concatenate([xu.reshape(b, c, n), rev], axis=1).reshape(
        b, 2 * c, h, w)
    catp = np.pad(cat, ((0, 0), (0, 0), (1, 1), (1, 1)))
    out = np.zeros((b, c, h, w), np.float32)
    for ky in range(3):
        for kx in range(3):
            out += np.einsum("oi,bihw->bohw", Wc[:, :, ky, kx],
                             catp[:, :, ky:ky + h, kx:kx + w])
    out += bc[None, :, None, None]
    return np.where(out >= 0, out, NEG * out).astype(np.float32)


def _run(in_maps, reps=1):
    nc = _get_prog(reps)
    return run_bass_kernel_spmd(nc, in_maps, list(range(NCORES)))


def kernel(inp, x, y, Wq, bq, Wk, bk, Wv, bv, Wc, bc, alpha, beta):
    args = [np.asarray(a, np.float32) for a in
            (inp, x, y, Wq, bq, Wk, bk, Wv, bv, Wc, bc)]
    alpha = np.asarray(alpha, np.float32)
    beta = np.asarray(beta, np.float32)
    if float(alpha.reshape(-1)[0]) != 1.0 or float(beta.reshape(-1)[0]) != 1.0:
        return _np_reference(*args, alpha.reshape(-1)[0], beta.reshape(-1)[0])
    in_maps = _prep_in_maps(*args)
    res = _run(in_maps)
    return _assemble(res.results)
